# revision 6
# baseline (speedup 1.0000x reference)
"""WaveNet-style gated residual conv layer on 8 Trainium2 NeuronCores.

Sharding: data-parallel over batch (B=8 -> 1 batch element per core).

Channel-major layout: one PSUM column per sequence position holds all
128 gate pre-activations (rows 0:64 = tanh-half y_t, rows 64:128 =
0.5 * sigmoid-half y_s; the sigmoid-half conv/cond weights and bias are
pre-scaled by 0.5 host-side).  Because sigmoid(y) = 0.5 + 0.5*tanh(y/2),
a SINGLE Tanh activation over all 128 partitions produces a = tanh(y_t)
and b = tanh(y_s/2); the gate z = a*sigmoid(y_s) = 0.5*a*(1+b).  zraw =
(b + 1) * a is ONE DVE scalar_tensor_tensor op; the 0.5 is folded into
the output weights (device) and the skip unpack (host).  The BIR
verifier requires equal base partitions for SBUF+SBUF input pairs, so b
is first copied to partitions 0:64 (DVE tensor_copy runs at 4x for
packed fp16, so this is cheap); cross-base *outputs* are legal, which
lets zraw land on either partition half of the pair-packed z tile.

Matmul cost on TRN2 is (output free width) x (cycles/row), independent
of contraction depth, so y is computed in 3 matmuls per 512-col chunk
(vs 5 naive):
  m1: K=128  [tap2 x(t)   ; cond ch 0:64 ]   (tile XC, window +16)
  m2: K= 80  [tap1 x(t-8) ; cond ch 64:80]   (tile XS, window +0)
  m3: K= 64  [tap0 x(t-16)]                  (tile XC, window +0)
XC rows 0:64 = x window (host left-padded 16), rows 64:128 = cond
channels 0:64 loaded 16 columns later so both align at one moving
window.  XS rows 0:64 = 8-column-shifted copy of x made on-chip by the
Pool engine (GPSIMD has no PSUM port but SBUF->SBUF tensor_copy is
fine, and Pool is otherwise idle); rows 64:80 = cond channels 64:80.

The 1x1 out-transform is pair-packed: zraw for two cells lands on
partition halves 0:64/64:128 of a shared z tile and one matmul with
blockdiag(0.5*Wout^T) produces both cells' outputs at once (0.5
passes/position).  All four out-matmuls of a window accumulate into one
[128,2048] PSUM tile flushed by a single Act Identity(+bias_out).

Per-core steady state per 512 positions: PE 1792 rows = 747ns, DMA
~775ns (17.9MB fp16 / 360GB/s -> the memory roofline), Act ~756ns,
DVE ~730ns, Pool ~724ns.  All HBM I/O fp16, fp32 PSUM accumulation.
"""

import numpy as np
from contextlib import ExitStack

import concourse.bass as bass
import concourse.tile as tile
from concourse import bacc, mybir
from concourse.bass_utils import run_bass_kernel_spmd

B, C_IN, T = 8, 64, 32768
R, KS, DIL, C_COND = 64, 3, 8, 80
PAD = (KS - 1) * DIL          # 16
W = 4096                      # window = DMA granularity
NW = T // W                   # 8
CELL = 1024                   # activation/psum cell (2 PSUM banks)
CHUNK = 512                   # matmul free width (1 PSUM bank fp32)
F32 = mybir.dt.float32
F16 = mybir.dt.float16
N_CORES = 8
AF = mybir.ActivationFunctionType
ALU = mybir.AluOpType

_cache = {}


def build_module():
    nc = bacc.Bacc(
        "TRN2", target_bir_lowering=False, debug=False, num_devices=N_CORES
    )

    xh = nc.dram_tensor("xh", [64, T + PAD], F16, kind="ExternalInput")
    ch = nc.dram_tensor("ch", [80, T], F16, kind="ExternalInput")
    ws = nc.dram_tensor("ws", [128, 3 * 128], F16, kind="ExternalInput")
    wo2 = nc.dram_tensor("wo2", [128, 128], F16, kind="ExternalInput")
    b3 = nc.dram_tensor("b3", [128, 2], F32, kind="ExternalInput")
    sk = nc.dram_tensor("sk", [128, T // 2], F16, kind="ExternalOutput")
    oh = nc.dram_tensor("oh", [128, T // 2], F16, kind="ExternalOutput")

    with tile.TileContext(nc) as tc, ExitStack() as ctx:
        const = ctx.enter_context(tc.tile_pool(name="const", bufs=1))
        xcpool = ctx.enter_context(tc.tile_pool(name="xc", bufs=2))
        xspool = ctx.enter_context(tc.tile_pool(name="xs", bufs=2))
        thpool = ctx.enter_context(tc.tile_pool(name="th", bufs=2))
        bcpool = ctx.enter_context(tc.tile_pool(name="bc", bufs=2))
        zpool = ctx.enter_context(tc.tile_pool(name="z", bufs=2))
        obpool = ctx.enter_context(tc.tile_pool(name="ob", bufs=2))
        ypool = ctx.enter_context(
            tc.tile_pool(name="y", bufs=2, space=bass.MemorySpace.PSUM)
        )
        oppool = ctx.enter_context(
            tc.tile_pool(name="op", bufs=1, space=bass.MemorySpace.PSUM)
        )

        w_sb = const.tile([128, 3 * 128], F16)
        wo_sb = const.tile([128, 128], F16)
        b_sb = const.tile([128, 2], F32)

        # --- prologue: PE p-state warm-up (the cost model reaches full
        # clock only after ~3us of continuous PE execution) on zero
        # matmuls while the first loads land; warm psum reuses the
        # (bufs=1) out-transform pool so no extra PSUM bank is needed ---
        warm = const.tile([128, CHUNK], F16)
        nc.vector.memset(warm[:, 0:256], 0.0)
        nc.vector.memset(warm[:, 256:], 0.0)
        wps = ypool.tile([128, CELL], F32, tag="yt")
        nc.tensor.matmul(wps[:, 0:256], warm[:, 0:128], warm[:, 0:256],
                         start=True, stop=True)
        nc.tensor.matmul(wps[:, 0:448], warm[:, 0:128], warm[:, 0:448],
                         start=True, stop=True)
        nc.tensor.matmul(wps[:, 0:480], warm[:, 0:128], warm[:, 0:480],
                         start=True, stop=True)

        xc_t = [None] * NW
        xs_t = [None] * NW

        def emit_loads(wj, pieces):
            """Load window wj.  pieces = list of (lo, hi) window-local
            column ranges (multiples of 512 except the end)."""
            c0 = wj * W
            xc = xcpool.tile([128, W + PAD], F16)
            xs = xspool.tile([128, W + 8], F16)
            xc_t[wj], xs_t[wj] = xc, xs
            cprev = 0
            for (lo, hi) in pieces:
                xlo, xhi = lo, (hi + PAD if hi == W else hi)
                nc.sync.dma_start(xc[0:64, xlo:xhi], xh[:, c0 + xlo:c0 + xhi])
                nc.sync.dma_start(xc[64:128, PAD + lo:PAD + hi],
                                  ch[0:64, c0 + lo:c0 + hi])
                nc.sync.dma_start(xs[64:80, lo:hi],
                                  ch[64:80, c0 + lo:c0 + hi])
                # 8-shifted x copy for tap1 (Pool, SBUF->SBUF); the copy
                # reads 8 columns ahead in xc, so it lags 8 columns
                # behind this piece's x load unless this is the last one
                cl, chi = cprev, (hi + 8 if hi == W else hi - 8)
                nc.gpsimd.tensor_copy(xs[0:64, cl:chi], xc[0:64, cl + 8:chi + 8])
                cprev = chi

        # out-transform matmuls deferred one cell so the PE never waits
        # on Act/DVE to produce z
        pending = []

        def drain_pending():
            for (zt, xblk, op) in pending:
                for q in (0, CHUNK):
                    off = CELL * xblk + q
                    nc.tensor.matmul(op[:, off:off + CHUNK], wo_sb[:, :],
                                     zt[:, off:off + CHUNK],
                                     start=True, stop=True)
            pending.clear()

        # first-window loads in two pieces (first cell's data lands
        # fast); weights first
        nc.sync.dma_start(w_sb[:, :], ws[:, :])
        nc.sync.dma_start(b_sb[:, :], b3[:, :])
        emit_loads(0, [(0, CELL), (CELL, W)])
        nc.sync.dma_start(wo_sb[:, :], wo2[:, :])

        zt = op = None
        flushes = []
        for g in range(NW * 4):            # global cell index
            wj, c = divmod(g, 4)
            if c == 0:
                if wj + 1 < NW:
                    emit_loads(wj + 1, [(0, W)])
                zt_prev, op_prev = zt, op
                zt = zpool.tile([128, W // 2], F16)
                op = oppool.tile([128, W // 2], F32)
            xc, xs = xc_t[wj], xs_t[wj]

            yt = ypool.tile([128, CELL], F32, tag="yt")
            for q in (0, CHUNK):
                base = c * CELL + q
                nc.tensor.matmul(yt[:, q:q + CHUNK], w_sb[:, 0:128],
                                 xc[:, base + PAD:base + PAD + CHUNK],
                                 start=True, stop=False)
                nc.tensor.matmul(yt[:, q:q + CHUNK], w_sb[0:80, 128:256],
                                 xs[0:80, base:base + CHUNK],
                                 start=False, stop=False)
                nc.tensor.matmul(yt[:, q:q + CHUNK], w_sb[0:64, 256:384],
                                 xc[0:64, base:base + CHUNK],
                                 start=False, stop=True)
            drain_pending()
            th = thpool.tile([128, CELL], F16)
            nc.scalar.activation(th[:, :], yt[:, :], AF.Tanh, bias=b_sb[:, 0:1])
            # flush + stores of the previous window, after this cell's
            # tanh on the Act queue (deps land earlier than queue turn)
            if c == 0 and wj > 0:
                ob = obpool.tile([128, W // 2], F16)
                nc.scalar.activation(ob[:, :], op_prev[:, :], AF.Identity,
                                     bias=b_sb[:, 1:2])
                nc.sync.dma_start(
                    sk[:, (wj - 1) * (W // 2):wj * (W // 2)], zt_prev[:, :])
                nc.sync.dma_start(
                    oh[:, (wj - 1) * (W // 2):wj * (W // 2)], ob[:, :])
            bc = bcpool.tile([64, CELL], F16)
            nc.vector.tensor_copy(bc[:, :], th[64:128, :])
            ph, xblk = c % 2, c // 2
            nc.vector.scalar_tensor_tensor(
                zt[64 * ph:64 * ph + 64, CELL * xblk:CELL * xblk + CELL],
                bc[:, :], 1.0, th[0:64, :], ALU.add, ALU.mult,
            )
            if ph == 1:
                pending.append((zt, xblk, op))
        drain_pending()
        ob = obpool.tile([128, W // 2], F16)
        nc.scalar.activation(ob[:, :], op[:, :], AF.Identity, bias=b_sb[:, 1:2])
        nc.sync.dma_start(sk[:, (NW - 1) * (W // 2):], zt[:, :])
        nc.sync.dma_start(oh[:, (NW - 1) * (W // 2):], ob[:, :])

    nc.compile()
    return nc


def pack_weights(weight_conv, bias_conv, weight_out, bias_out, weight_cond):
    wc3 = weight_conv.astype(np.float32)           # [128, 64, 3]
    wcd = weight_cond[:, :, 0].astype(np.float32)  # [128, 80]
    scale = np.ones((128, 1), np.float32)
    scale[64:] = 0.5                               # sigmoid half: tanh(y/2)
    S = np.zeros((128, 3 * 128), np.float32)
    # m1: tap2 + cond 0:64
    S[0:64, 0:128] = (wc3[:, :, 2] * scale).T
    S[64:128, 0:128] = (wcd[:, 0:64] * scale).T
    # m2: tap1 + cond 64:80
    S[0:64, 128:256] = (wc3[:, :, 1] * scale).T
    S[64:80, 128:256] = (wcd[:, 64:80] * scale).T
    # m3: tap0
    S[0:64, 256:384] = (wc3[:, :, 0] * scale).T
    wo2 = np.zeros((128, 128), np.float32)
    woT = 0.5 * weight_out[:, :, 0].astype(np.float32).T   # zraw = 2z
    wo2[0:64, 0:64] = woT
    wo2[64:128, 64:128] = woT
    b3 = np.zeros((128, 2), np.float32)
    b3[0:64, 0] = bias_conv[0:64]
    b3[64:128, 0] = 0.5 * bias_conv[64:128]
    b3[0:64, 1] = bias_out
    b3[64:128, 1] = bias_out
    return S.astype(np.float16), wo2.astype(np.float16), b3


def make_in_maps(x, cond, weight_conv, bias_conv, weight_out, bias_out,
                 weight_cond):
    S, wo2, b3 = pack_weights(weight_conv, bias_conv, weight_out, bias_out,
                              weight_cond)
    pad = np.zeros((64, PAD), np.float16)
    in_maps = []
    for b in range(B):
        xhb = np.concatenate([pad, x[b].astype(np.float16)], axis=1)
        in_maps.append({
            "xh": np.ascontiguousarray(xhb),
            "ch": np.ascontiguousarray(cond[b].astype(np.float16)),
            "ws": S, "wo2": wo2, "b3": b3,
        })
    return in_maps


def _unpack(a2):
    # [128, T/2] -> [64, T]; partition p = 64*ph + chan; col =
    # 2048*win + 1024*xblk + n; t = 4096*win + 2048*xblk + 1024*ph + n
    a = a2.astype(np.float32).reshape(2, 64, NW, 2, CELL)
    return a.transpose(1, 2, 3, 0, 4).reshape(64, T)


def unpack_outputs(results):
    output = np.empty((B, R, T), np.float32)
    skip = np.empty((B, R, T), np.float32)
    for b in range(B):
        output[b] = _unpack(results[b]["oh"])
        skip[b] = _unpack(results[b]["sk"]) * 0.5    # z = 0.5 * zraw
    return output, skip


def kernel(**inputs):
    inputs = {k: np.asarray(v, dtype=np.float32) for k, v in inputs.items()}
    if "nc" not in _cache:
        _cache["nc"] = build_module()
    nc = _cache["nc"]
    in_maps = make_in_maps(**inputs)
    res = run_bass_kernel_spmd(nc, in_maps, list(range(N_CORES)))
    return unpack_outputs(res.results)


# revision 10
# speedup vs baseline: 1.2935x; 1.2935x over previous
"""WaveNet-style gated residual conv layer on 8 Trainium2 NeuronCores.

Sharding: data-parallel over batch (B=8 -> 1 batch element per core).

Channel-major layout: one PSUM column per sequence position holds all
128 gate pre-activations (rows 0:64 = tanh-half y_t, rows 64:128 =
0.5 * sigmoid-half y_s; the sigmoid-half conv/cond weights and bias are
pre-scaled by 0.5 host-side).  Because sigmoid(y) = 0.5 + 0.5*tanh(y/2),
a SINGLE Tanh activation over all 128 partitions produces a = tanh(y_t)
and b = tanh(y_s/2); the gate z = a*sigmoid(y_s) = 0.5*a*(1+b).  zraw =
(b + 1) * a is ONE DVE scalar_tensor_tensor op; the 0.5 is folded into
the output weights (device) and the skip unpack (host).  The BIR
verifier requires equal base partitions for SBUF+SBUF input pairs, so b
is first copied to partitions 0:64 (DVE tensor_copy runs at 4x for
packed fp16, so this is cheap); cross-base *outputs* are legal, which
lets zraw land on either partition half of the pair-packed z tile.

Matmul cost on TRN2 is (output free width) x (cycles/row), independent
of contraction depth, so y is computed in 3 matmuls per 512-col chunk
(vs 5 naive):
  m1: K=128  [tap2 x(t)   ; cond ch 0:64 ]   (tile XC, window +16)
  m2: K= 80  [tap1 x(t-8) ; cond ch 64:80]   (tile XS, window +0)
  m3: K= 64  [tap0 x(t-16)]                  (tile XC, window +0)
XC rows 0:64 = x window (host left-padded 16), rows 64:128 = cond
channels 0:64 loaded 16 columns later so both align at one moving
window.  XS rows 0:64 = 8-column-shifted copy of x made on-chip by the
Pool engine (GPSIMD has no PSUM port but SBUF->SBUF tensor_copy is
fine, and Pool is otherwise idle); rows 64:80 = cond channels 64:80.

The 1x1 out-transform is pair-packed: zraw for two cells lands on
partition halves 0:64/64:128 of a shared z tile and one matmul with
blockdiag(0.5*Wout^T) produces both cells' outputs at once (0.5
passes/position).  All four out-matmuls of a window accumulate into one
[128,2048] PSUM tile flushed by a single Act Identity(+bias_out).

Per-core steady state per 512 positions: PE 1792 rows = 747ns, DMA
~775ns (17.9MB fp16 / 360GB/s -> the memory roofline), Act ~756ns,
DVE ~730ns, Pool ~724ns.  All HBM I/O fp16, fp32 PSUM accumulation.
"""

import numpy as np
from contextlib import ExitStack

import concourse.bass as bass
import concourse.tile as tile
from concourse import bacc, mybir
from concourse.bass_utils import run_bass_kernel_spmd

B, C_IN, T = 8, 64, 32768
R, KS, DIL, C_COND = 64, 3, 8, 80
PAD = (KS - 1) * DIL          # 16
W = 4096                      # window = DMA granularity
NW = T // W                   # 8
CELL = 1024                   # activation/psum cell (2 PSUM banks)
CHUNK = 512                   # matmul free width (1 PSUM bank fp32)
F32 = mybir.dt.float32
F16 = mybir.dt.float16
N_CORES = 8
AF = mybir.ActivationFunctionType
ALU = mybir.AluOpType

_cache = {}


def build_module():
    nc = bacc.Bacc(
        "TRN2", target_bir_lowering=False, debug=False, num_devices=N_CORES
    )

    # xch rows 0:64 = [16 zeros, x]; rows 64:128 = [16 zeros, cond 0:64]
    # so ONE DMA per window loads both x and cond_lo with the relative
    # 16-column shift the m1 matmul window expects baked in host-side.
    xch = nc.dram_tensor("xch", [128, T + PAD], F16, kind="ExternalInput")
    chi = nc.dram_tensor("chi", [16, T], F16, kind="ExternalInput")
    ws = nc.dram_tensor("ws", [128, 3 * 128], F16, kind="ExternalInput")
    wo2 = nc.dram_tensor("wo2", [128, 128], F16, kind="ExternalInput")
    b3 = nc.dram_tensor("b3", [128, 2], F32, kind="ExternalInput")
    sk = nc.dram_tensor("sk", [128, T // 2], F16, kind="ExternalOutput")
    oh = nc.dram_tensor("oh", [128, T // 2], F16, kind="ExternalOutput")

    with tile.TileContext(nc) as tc, ExitStack() as ctx:
        const = ctx.enter_context(tc.tile_pool(name="const", bufs=1))
        xcpool = ctx.enter_context(tc.tile_pool(name="xc", bufs=2))
        xspool = ctx.enter_context(tc.tile_pool(name="xs", bufs=2))
        thpool = ctx.enter_context(tc.tile_pool(name="th", bufs=2))
        bcpool = ctx.enter_context(tc.tile_pool(name="bc", bufs=2))
        zpool = ctx.enter_context(tc.tile_pool(name="z", bufs=2))
        obpool = ctx.enter_context(tc.tile_pool(name="ob", bufs=2))
        ypool = ctx.enter_context(
            tc.tile_pool(name="y", bufs=2, space=bass.MemorySpace.PSUM)
        )
        oppool = ctx.enter_context(
            tc.tile_pool(name="op", bufs=1, space=bass.MemorySpace.PSUM)
        )

        w_sb = const.tile([128, 3 * 128], F16)
        wo_sb = const.tile([128, 128], F16)
        b_sb = const.tile([128, 2], F32)

        # --- prologue: PE p-state warm-up (the cost model reaches full
        # clock only after ~3us of continuous PE execution) on zero
        # matmuls while the first loads land; warm psum reuses the
        # (bufs=1) out-transform pool so no extra PSUM bank is needed ---
        warm = const.tile([128, CHUNK], F16)
        nc.vector.memset(warm[:, 0:256], 0.0)
        nc.vector.memset(warm[:, 256:], 0.0)
        wps = ypool.tile([128, CELL], F32, tag="yt")
        nc.tensor.matmul(wps[:, 0:256], warm[:, 0:128], warm[:, 0:256],
                         start=True, stop=True)
        nc.tensor.matmul(wps[:, 0:448], warm[:, 0:128], warm[:, 0:448],
                         start=True, stop=True)
        nc.tensor.matmul(wps[:, 0:480], warm[:, 0:128], warm[:, 0:480],
                         start=True, stop=True)

        xc_t = [None] * NW
        xs_t = [None] * NW

        def emit_loads(wj, pieces):
            """Load window wj.  pieces = list of (lo, hi) window-local
            column ranges (multiples of 512 except the end)."""
            c0 = wj * W
            xc = xcpool.tile([128, W + PAD], F16)
            xs = xspool.tile([128, W + 8], F16)
            xc_t[wj], xs_t[wj] = xc, xs
            cprev = 0
            for (lo, hi) in pieces:
                xlo, xhi = lo, (hi + PAD if hi == W else hi)
                nc.sync.dma_start(xc[:, xlo:xhi], xch[:, c0 + xlo:c0 + xhi])
                nc.sync.dma_start(xs[64:80, lo:hi],
                                  chi[:, c0 + lo:c0 + hi])
                # 8-shifted x copy for tap1 (Pool, SBUF->SBUF); the copy
                # reads 8 columns ahead in xc, so it lags 8 columns
                # behind this piece's x load unless this is the last one
                cl, chh = cprev, (hi + 8 if hi == W else hi - 8)
                if wj > 0 and len(pieces) == 1:
                    # split so the first half is ready before the window
                    # starts (a full-width copy finishes ~2.5us too late)
                    mid = W // 2
                    nc.gpsimd.tensor_copy(xs[0:64, cl:mid],
                                          xc[0:64, cl + 8:mid + 8])
                    nc.gpsimd.tensor_copy(xs[0:64, mid:chh],
                                          xc[0:64, mid + 8:chh + 8])
                else:
                    nc.gpsimd.tensor_copy(xs[0:64, cl:chh],
                                          xc[0:64, cl + 8:chh + 8])
                cprev = chh

        # out-transform matmuls deferred one cell so the PE never waits
        # on Act/DVE to produce z
        pending = []

        def drain_pending():
            for (zt, xblk, op) in pending:
                for q in (0, CHUNK):
                    off = CELL * xblk + q
                    nc.tensor.matmul(op[:, off:off + CHUNK], wo_sb[:, :],
                                     zt[:, off:off + CHUNK],
                                     start=True, stop=True)
            pending.clear()

        # first-window loads in two pieces (first cell's data lands
        # fast); weights first
        nc.sync.dma_start(w_sb[:, :], ws[:, :])
        nc.sync.dma_start(b_sb[:, :], b3[:, :])
        emit_loads(0, [(0, CELL + CHUNK), (CELL + CHUNK, W)])
        nc.sync.dma_start(wo_sb[:, :], wo2[:, :])

        zt = op = None
        flushes = []
        for g in range(NW * 4):            # global cell index
            wj, c = divmod(g, 4)
            if c == 0:
                if wj + 1 < NW:
                    emit_loads(wj + 1, [(0, W)])
                zt_prev, op_prev = zt, op
                zt = zpool.tile([128, W // 2], F16)
                op = oppool.tile([128, W // 2], F32)
            xc, xs = xc_t[wj], xs_t[wj]

            yt = ypool.tile([128, CELL], F32, tag="yt")
            for q in (0, CHUNK):
                base = c * CELL + q
                nc.tensor.matmul(yt[:, q:q + CHUNK], w_sb[:, 0:128],
                                 xc[:, base + PAD:base + PAD + CHUNK],
                                 start=True, stop=False)
                nc.tensor.matmul(yt[:, q:q + CHUNK], w_sb[0:80, 128:256],
                                 xs[0:80, base:base + CHUNK],
                                 start=False, stop=False)
                nc.tensor.matmul(yt[:, q:q + CHUNK], w_sb[0:64, 256:384],
                                 xc[0:64, base:base + CHUNK],
                                 start=False, stop=True)
            drain_pending()
            th = thpool.tile([128, CELL], F16)
            nc.scalar.activation(th[:, :], yt[:, :], AF.Tanh, bias=b_sb[:, 0:1])
            # flush + stores of the previous window, after this cell's
            # tanh on the Act queue (deps land earlier than queue turn)
            if c == 0 and wj > 0:
                ob = obpool.tile([128, W // 2], F16)
                nc.scalar.activation(ob[:, :], op_prev[:, :], AF.Identity,
                                     bias=b_sb[:, 1:2])
                nc.sync.dma_start(
                    sk[:, (wj - 1) * (W // 2):wj * (W // 2)], zt_prev[:, :])
                nc.sync.dma_start(
                    oh[:, (wj - 1) * (W // 2):wj * (W // 2)], ob[:, :])
            bc = bcpool.tile([64, CELL], F16)
            nc.vector.tensor_copy(bc[:, :], th[64:128, :])
            ph, xblk = c % 2, c // 2
            nc.vector.scalar_tensor_tensor(
                zt[64 * ph:64 * ph + 64, CELL * xblk:CELL * xblk + CELL],
                bc[:, :], 1.0, th[0:64, :], ALU.add, ALU.mult,
            )
            if ph == 1:
                pending.append((zt, xblk, op))
        drain_pending()
        ob = obpool.tile([128, W // 2], F16)
        nc.scalar.activation(ob[:, :], op[:, :], AF.Identity, bias=b_sb[:, 1:2])
        nc.sync.dma_start(sk[:, (NW - 1) * (W // 2):], zt[:, :])
        nc.sync.dma_start(oh[:, (NW - 1) * (W // 2):], ob[:, :])

    nc.compile()
    return nc


def pack_weights(weight_conv, bias_conv, weight_out, bias_out, weight_cond):
    wc3 = weight_conv.astype(np.float32)           # [128, 64, 3]
    wcd = weight_cond[:, :, 0].astype(np.float32)  # [128, 80]
    scale = np.ones((128, 1), np.float32)
    scale[64:] = 0.5                               # sigmoid half: tanh(y/2)
    S = np.zeros((128, 3 * 128), np.float32)
    # m1: tap2 + cond 0:64
    S[0:64, 0:128] = (wc3[:, :, 2] * scale).T
    S[64:128, 0:128] = (wcd[:, 0:64] * scale).T
    # m2: tap1 + cond 64:80
    S[0:64, 128:256] = (wc3[:, :, 1] * scale).T
    S[64:80, 128:256] = (wcd[:, 64:80] * scale).T
    # m3: tap0
    S[0:64, 256:384] = (wc3[:, :, 0] * scale).T
    wo2 = np.zeros((128, 128), np.float32)
    woT = 0.5 * weight_out[:, :, 0].astype(np.float32).T   # zraw = 2z
    wo2[0:64, 0:64] = woT
    wo2[64:128, 64:128] = woT
    b3 = np.zeros((128, 2), np.float32)
    b3[0:64, 0] = bias_conv[0:64]
    b3[64:128, 0] = 0.5 * bias_conv[64:128]
    b3[0:64, 1] = bias_out
    b3[64:128, 1] = bias_out
    return S.astype(np.float16), wo2.astype(np.float16), b3


def make_in_maps(x, cond, weight_conv, bias_conv, weight_out, bias_out,
                 weight_cond):
    S, wo2, b3 = pack_weights(weight_conv, bias_conv, weight_out, bias_out,
                              weight_cond)
    pad = np.zeros((128, PAD), np.float16)
    in_maps = []
    for b in range(B):
        body = np.concatenate(
            [x[b].astype(np.float16), cond[b, 0:64].astype(np.float16)], axis=0)
        xchb = np.concatenate([pad, body], axis=1)
        in_maps.append({
            "xch": np.ascontiguousarray(xchb),
            "chi": np.ascontiguousarray(cond[b, 64:80].astype(np.float16)),
            "ws": S, "wo2": wo2, "b3": b3,
        })
    return in_maps


def _unpack(a2):
    # [128, T/2] -> [64, T]; partition p = 64*ph + chan; col =
    # 2048*win + 1024*xblk + n; t = 4096*win + 2048*xblk + 1024*ph + n
    a = a2.astype(np.float32).reshape(2, 64, NW, 2, CELL)
    return a.transpose(1, 2, 3, 0, 4).reshape(64, T)


def unpack_outputs(results):
    output = np.empty((B, R, T), np.float32)
    skip = np.empty((B, R, T), np.float32)
    for b in range(B):
        output[b] = _unpack(results[b]["oh"])
        skip[b] = _unpack(results[b]["sk"]) * 0.5    # z = 0.5 * zraw
    return output, skip


def kernel(**inputs):
    inputs = {k: np.asarray(v, dtype=np.float32) for k, v in inputs.items()}
    if "nc" not in _cache:
        _cache["nc"] = build_module()
    nc = _cache["nc"]
    in_maps = make_in_maps(**inputs)
    res = run_bass_kernel_spmd(nc, in_maps, list(range(N_CORES)))
    return unpack_outputs(res.results)
